# revision 3
# baseline (speedup 1.0000x reference)
"""Trainium2 Bass kernel for nn_Damping: per-sample Cholesky-factor damping.

Math (per sample b):
  h  = tanh MLPs of x0 -> diag xd [64], offdiag z [2016] (strict lower tri of L)
  y  = L^T x0 ; D = L y

Implementation (per core, feature-major layout [feature partitions, batch free]):
  - all matmuls in fp32r (full PE rate at free>=256, ~1e-4 rel err)
  - L matvecs without materializing L, using static 0/1 scatter/gather
    matrices on the tensor engine:
      x0g = R @ x0          (x0g[p] = x0[row(p)])
      y   = C^T (z*x0g) + Q1 x0 + xd*x0      (Q1 = C^T diag(boo) R)
      yg  = C @ y
      D   = R^T (z*yg) + Q2 y + xd*y         (Q2 = R^T diag(boo) C)
    (boo never touches the device; folded into Q1/Q2 on host)

Data parallel over 8 cores: batch 32768 -> 8 x 4096.
"""

import sys

if "/opt/trn_rl_repo" not in sys.path:
    sys.path.insert(0, "/opt/trn_rl_repo")

import numpy as np

N = 64
H = 256
B = 32768
OFF = 2016
NCORES = 8
B_CORE = B // NCORES   # 4096
F = 512                # batch tile (free dim)
NCHUNK = 16            # 2016 = 16 * 126
CH = OFF // NCHUNK     # 126


def _build_nc(b_core=B_CORE, f=F):
    import concourse.bacc as bacc
    import concourse.mybir as mybir
    import concourse.tile as tile

    F32 = mybir.dt.float32
    F32R = mybir.dt.float32r
    Tanh = mybir.ActivationFunctionType.Tanh
    Copy = mybir.ActivationFunctionType.Copy

    ntiles = b_core // f
    assert b_core % f == 0 and f % 128 == 0
    ncol = f // 128

    nc = bacc.Bacc("TRN2", target_bir_lowering=False, debug=False,
                   num_devices=NCORES)

    # --- DRAM tensors ---
    x_d = nc.dram_tensor("x", [b_core, N], F32R, kind="ExternalInput")
    wd1_d = nc.dram_tensor("wd1t", [N, H], F32R, kind="ExternalInput")
    wd2_d = nc.dram_tensor("wd2t", [H, H], F32R, kind="ExternalInput")
    wdo_d = nc.dram_tensor("wdot", [H, N], F32R, kind="ExternalInput")
    wo1_d = nc.dram_tensor("wo1t", [N, H], F32R, kind="ExternalInput")
    wo2_d = nc.dram_tensor("wo2t", [H, H], F32R, kind="ExternalInput")
    woo_d = nc.dram_tensor("woot", [H, OFF], F32R, kind="ExternalInput")
    r_d = nc.dram_tensor("rmat", [OFF, N], F32R, kind="ExternalInput")
    c_d = nc.dram_tensor("cmat", [OFF, N], F32R, kind="ExternalInput")
    rt_d = nc.dram_tensor("rtmat", [N, OFF], F32R, kind="ExternalInput")
    ct_d = nc.dram_tensor("ctmat", [N, OFF], F32R, kind="ExternalInput")
    bl_d = nc.dram_tensor("blmat", [N, N], F32R, kind="ExternalInput")
    blt_d = nc.dram_tensor("bltmat", [N, N], F32R, kind="ExternalInput")
    id_d = nc.dram_tensor("ident", [128, 128], F32R, kind="ExternalInput")
    bd1_d = nc.dram_tensor("bd1", [2, 128, 1], F32, kind="ExternalInput")
    bd2_d = nc.dram_tensor("bd2", [2, 128, 1], F32, kind="ExternalInput")
    bo1_d = nc.dram_tensor("bo1", [2, 128, 1], F32, kind="ExternalInput")
    bo2_d = nc.dram_tensor("bo2", [2, 128, 1], F32, kind="ExternalInput")
    bdo_d = nc.dram_tensor("bdo", [N, 1], F32, kind="ExternalInput")
    out_d = nc.dram_tensor("out", [b_core, N], F32, kind="ExternalOutput")

    with tile.TileContext(nc) as tc:
        with (
            tc.tile_pool(name="wpool", bufs=1) as wp,
            tc.tile_pool(name="apool", bufs=1) as ap,
            tc.tile_pool(name="zpool", bufs=1) as zp,
            tc.tile_pool(name="upool", bufs=1) as up,
            tc.tile_pool(name="iopool", bufs=1) as iop,
            tc.tile_pool(name="psum", bufs=1, space="PSUM") as pp,
        ):
            # ---- weight preload (once) ----
            def wtile(name, src, shape):
                t = wp.tile(shape, F32R, tag=name, name=name, bufs=1)
                nc.sync.dma_start(t[:], src)
                return t

            wd1 = wtile("wd1", wd1_d[:], [N, H])
            wd2 = [wtile(f"wd2_{k}", wd2_d[k * 128:(k + 1) * 128, :], [128, H])
                   for k in range(2)]
            wdo = [wtile(f"wdo_{k}", wdo_d[k * 128:(k + 1) * 128, :], [128, N])
                   for k in range(2)]
            wo1 = wtile("wo1", wo1_d[:], [N, H])
            wo2 = [wtile(f"wo2_{k}", wo2_d[k * 128:(k + 1) * 128, :], [128, H])
                   for k in range(2)]
            woo = [wtile(f"woo_{k}", woo_d[k * 128:(k + 1) * 128, :], [128, OFF])
                   for k in range(2)]
            rmat = [wtile(f"rm_{m}", r_d[m * CH:(m + 1) * CH, :], [CH, N])
                    for m in range(NCHUNK)]
            cmat = [wtile(f"cm_{m}", c_d[m * CH:(m + 1) * CH, :], [CH, N])
                    for m in range(NCHUNK)]
            rtm = wtile("rtm", rt_d[:], [N, OFF])
            ctm = wtile("ctm", ct_d[:], [N, OFF])
            blm = wtile("blm", bl_d[:], [N, N])
            bltm = wtile("bltm", blt_d[:], [N, N])
            ident = wtile("ident", id_d[:], [128, 128])

            def btile(name, src, p):
                t = wp.tile([p, 1], F32, tag=name, name=name, bufs=1)
                nc.sync.dma_start(t[:], src)
                return t

            bd1 = [btile(f"bd1_{k}", bd1_d[k], 128) for k in range(2)]
            bd2 = [btile(f"bd2_{k}", bd2_d[k], 128) for k in range(2)]
            bo1 = [btile(f"bo1_{k}", bo1_d[k], 128) for k in range(2)]
            bo2 = [btile(f"bo2_{k}", bo2_d[k], 128) for k in range(2)]
            bdo = btile("bdo", bdo_d[:], N)

            # ---- per batch-tile pipeline ----
            for t in range(ntiles):
                b0 = t * f
                # input + transpose -> x0T [64, f]
                x_in = iop.tile([128, ncol, N], F32R, tag="x_in", bufs=2)
                nc.sync.dma_start(
                    x_in[:],
                    x_d[b0:b0 + f, :].rearrange("(c p) n -> p c n", p=128))
                px = pp.tile([N, ncol, 128], F32R, tag="ptr", bufs=2)
                for c in range(ncol):
                    nc.tensor.transpose(px[:, c, :], x_in[:, c, :], ident[:])
                x0T = ap.tile([N, f], F32R, tag="x0T", bufs=2)
                nc.vector.tensor_copy(x0T[:], px.rearrange("p c n -> p (c n)"))

                # MLP hidden layers (tanh)
                def layer(tag, wts, rhss, biases, nout):
                    outs = []
                    for m in range(nout // 128):
                        ph = pp.tile([128, f], F32, tag="ph", bufs=2)
                        nk = len(wts)
                        for k in range(nk):
                            nc.tensor.matmul(
                                ph[:], wts[k][:, m * 128:(m + 1) * 128],
                                rhss[k][:], start=(k == 0), stop=(k == nk - 1))
                        h = ap.tile([128, f], F32R, tag=f"{tag}{m}", bufs=2)
                        nc.scalar.activation(h[:], ph[:], Tanh,
                                             bias=biases[m][:, 0:1])
                        outs.append(h)
                    return outs

                h1d = layer("h1d", [wd1], [x0T], bd1, H)
                h1o = layer("h1o", [wo1], [x0T], bo1, H)
                h2d = layer("h2d", wd2, h1d, bd2, H)
                h2o = layer("h2o", wo2, h1o, bo2, H)

                # diag head: xd = WdoT.T @ h2d + bdo
                pxd = pp.tile([N, f], F32, tag="ph", bufs=2)
                for k in range(2):
                    nc.tensor.matmul(pxd[:], wdo[k][:], h2d[k][:],
                                     start=(k == 0), stop=(k == 1))
                xd = ap.tile([N, f], F32, tag="xd", bufs=2)
                nc.vector.tensor_scalar_add(xd[:], pxd[:], bdo[:, 0:1])

                # offdiag head: z chunks [126, f] (no bias; folded into Q1/Q2)
                z_sb = []
                for m in range(NCHUNK):
                    pz = pp.tile([CH, f], F32, tag="ph", bufs=2)
                    for k in range(2):
                        nc.tensor.matmul(
                            pz[:], woo[k][:, m * CH:(m + 1) * CH], h2o[k][:],
                            start=(k == 0), stop=(k == 1))
                    zt = zp.tile([CH, f], F32R, tag=f"z{m}", bufs=2)
                    nc.scalar.activation(zt[:], pz[:], Copy)
                    z_sb.append(zt)

                # y = C^T(z * R x0) + Q1 x0 + xd*x0
                py = pp.tile([N, f], F32, tag="acc", bufs=2)
                nc.tensor.matmul(py[:], blm[:], x0T[:], start=True, stop=False,
                                 skip_group_check=True)
                for m in range(NCHUNK):
                    pg = pp.tile([CH, f], F32, tag="pg", bufs=2)
                    nc.tensor.matmul(pg[:], rtm[:, m * CH:(m + 1) * CH],
                                     x0T[:], start=True, stop=True)
                    u = up.tile([CH, f], F32R, tag="u", bufs=3)
                    nc.vector.tensor_mul(u[:], z_sb[m][:], pg[:])
                    nc.tensor.matmul(py[:], cmat[m][:], u[:], start=False,
                                     stop=(m == NCHUNK - 1),
                                     skip_group_check=True)
                t1 = ap.tile([N, f], F32, tag="t1", bufs=2)
                nc.vector.tensor_mul(t1[:], xd[:], x0T[:])
                y = ap.tile([N, f], F32R, tag="y", bufs=2)
                nc.vector.tensor_add(y[:], t1[:], py[:])

                # D = R^T(z * C y) + Q2 y + xd*y
                pd = pp.tile([N, f], F32, tag="acc", bufs=2)
                nc.tensor.matmul(pd[:], bltm[:], y[:], start=True, stop=False,
                                 skip_group_check=True)
                for m in range(NCHUNK):
                    pg2 = pp.tile([CH, f], F32, tag="pg", bufs=2)
                    nc.tensor.matmul(pg2[:], ctm[:, m * CH:(m + 1) * CH],
                                     y[:], start=True, stop=True)
                    v = up.tile([CH, f], F32R, tag="v", bufs=3)
                    nc.vector.tensor_mul(v[:], z_sb[m][:], pg2[:])
                    nc.tensor.matmul(pd[:], rmat[m][:], v[:], start=False,
                                     stop=(m == NCHUNK - 1),
                                     skip_group_check=True)
                t2 = ap.tile([N, f], F32, tag="t2", bufs=2)
                nc.vector.tensor_mul(t2[:], xd[:], y[:])
                dd = ap.tile([N, f], F32R, tag="dd", bufs=2)
                nc.vector.tensor_add(dd[:], t2[:], pd[:])

                # transpose back + store
                po = pp.tile([128, ncol, N], F32R, tag="ptr", bufs=2)
                for c in range(ncol):
                    nc.tensor.transpose(po[:, c, :], dd[:, c * 128:(c + 1) * 128],
                                        ident[:N, :N])
                o_sb = iop.tile([128, ncol, N], F32, tag="o_sb", bufs=2)
                nc.vector.tensor_copy(o_sb[:], po[:])
                nc.sync.dma_start(
                    out_d[b0:b0 + f, :].rearrange("(c p) n -> p c n", p=128),
                    o_sb[:])

    nc.compile()
    return nc


def _host_constants(Wd1, bd1, Wd2, bd2, Wdo, bdo, Wo1, bo1, Wo2, bo2, Woo, boo):
    """Shared (per-core replicated) input arrays."""
    f32 = np.float32
    rows, cols = np.tril_indices(N, k=-1)
    R = np.zeros((OFF, N), f32)
    R[np.arange(OFF), rows] = 1.0
    C = np.zeros((OFF, N), f32)
    C[np.arange(OFF), cols] = 1.0
    BL = np.zeros((N, N), f32)
    BL[rows, cols] = np.asarray(boo, f32)

    def ct(a):
        return np.ascontiguousarray(a, dtype=f32)

    return {
        "wd1t": ct(np.asarray(Wd1).T), "wd2t": ct(np.asarray(Wd2).T),
        "wdot": ct(np.asarray(Wdo).T), "wo1t": ct(np.asarray(Wo1).T),
        "wo2t": ct(np.asarray(Wo2).T), "woot": ct(np.asarray(Woo).T),
        "rmat": R, "cmat": C, "rtmat": ct(R.T), "ctmat": ct(C.T),
        "blmat": BL, "bltmat": ct(BL.T),
        "ident": np.eye(128, dtype=f32),
        "bd1": ct(np.asarray(bd1).reshape(2, 128, 1)),
        "bd2": ct(np.asarray(bd2).reshape(2, 128, 1)),
        "bo1": ct(np.asarray(bo1).reshape(2, 128, 1)),
        "bo2": ct(np.asarray(bo2).reshape(2, 128, 1)),
        "bdo": ct(np.asarray(bdo).reshape(N, 1)),
    }


_NC_CACHE = {}


def get_nc(b_core=B_CORE, f=F):
    key = (b_core, f)
    if key not in _NC_CACHE:
        _NC_CACHE[key] = _build_nc(b_core, f)
    return _NC_CACHE[key]


def make_in_maps(input, **params):
    shared = _host_constants(**params)
    x = np.ascontiguousarray(np.asarray(input), dtype=np.float32)
    assert x.shape == (B, N)
    return [dict(shared, x=x[c * B_CORE:(c + 1) * B_CORE]) for c in range(NCORES)]


def kernel(input, **params):
    from concourse import bass_utils

    nc = get_nc()
    in_maps = make_in_maps(input, **params)
    res = bass_utils.run_bass_kernel_spmd(nc, in_maps,
                                          core_ids=list(range(NCORES)))
    return np.concatenate([r["out"] for r in res.results], axis=0)
